# revision 51
# baseline (speedup 1.0000x reference)
"""Trainium2 Bass kernel for nn_MemoryModule (sparse_attention).

Reference computation (per batch b):
  Low branch:
    mkl (9216, 64) = memory_keys_low[b] as (T*Hl*Wl, Ck)
    qkl (64, 2304) = query_key_low[b]
    A = softmax_over_n(mkl @ qkl * Ck^-0.5)          # (9216, 2304)
    memory = mvl @ A                                  # (128, 2304), mvl = (Cv, T*Hl*Wl)
  High branch:
    g_attn[t] = softmax_over_t(gk[t] @ gv[t].T * Cv^-0.5)   # (Ck, Cv) per t
    qout[t] = g_attn[t] @ qv                          # (64, 576) -> (256, 24, 24)
    qout = bilinear_upsample_2x(qout)                 # (256, 48, 48)
  out = concat([qout, memory.reshape(128, 48, 48)])   # (384, 48, 48)

Sharding: 8 cores = (b in 0..1) x (j in 0..3), where j picks 576 of the 2304
low-branch query columns (= 12 of the 48 output rows). Softmax is over the
key axis, so column blocks are independent -> no collectives.

Design notes (evolved from the v1 pipeline via trace analysis):
 - QK lhsT is zero-padded from K=64 to K=128 so every matmul in the kernel
   runs in the same (128,128) PE mode: no row-group tiling mode switches,
   and the compiler's Fast Weight Load stays enabled for the QK stream.
 - The softmax denominator main part (512 cols) runs on the PE as
   DoubleRow ones-matmuls, each emitted directly behind its exp: this
   spreads the ballast into the first half of the loop (where the PE has
   spare capacity) and keeps the HAM clock gate warm from ~16us on
   (cold PE = 1.2 GHz, warm = 2.4 GHz). Only the runt part (64 cols)
   accumulates on the DVE (9 contiguous adds); elementwise engines
   measured 1.2-3 ns/elem - far too slow for the 37k-element main sum.
 - The attention matrix is stored as TWO tensors, e_main (128,72,512)
   and e_runt (128,72,64), so every exp write, every AV DoubleRow rhs
   and the DVE accumulation slices are fully contiguous.
 - exp() writes fp8-e4m3 (constant logit shift -2.5 cancels in softmax).
 - QK results land in PSUM as pair chunks (128,2,512) -> one activation
   covers 1024 elements (ACT costs (N+352)/1.2 ns, so batching matters);
   64-col leftovers of 8 tiles share one bank (runt matmuls trail the
   mains by 4 tiles so bufs=1 never stalls).
 - QK mains are emitted under tc.high_priority(): the scheduler's cost
   model underestimates the DR consume stream and otherwise bunches it
   ahead of the mains, starving the exp pipeline for ~6us.
 - The runt-group-8 chain (QK runts, exp, DVE folds, partition-fold
   matmul, runt reciprocal) is pulled into the loop at q=70, ahead of
   the last main exp, so it overlaps the dn/av drains. The fold lands
   in the freed runts bank - sharing a PSUM tile with the a64 AV chain
   would serialize the drain behind the reciprocal reader (~1.5us).
 - AV consumes fp8 n-tile PAIRS via MatmulPerfMode.DoubleRow (K=256).
 - PSUM budget stays exactly 8 banks in every phase:
   mains 2x2 + runts 1 + dn 1 + {hps 1 | qvups 2 | qops 2 | av 1 + a64 1}.
"""

import os
import sys

for _p in ("/opt/trn_rl_repo",):
    if _p not in sys.path and os.path.isdir(_p):
        sys.path.insert(0, _p)

import numpy as np
import ml_dtypes

import concourse.bass as bass
import concourse.tile as tile
from concourse import bacc, mybir
from concourse import bass_utils

BF16 = mybir.dt.bfloat16
F32 = mybir.dt.float32
F8 = mybir.dt.float8e4

B, T, Ck, Cv = 2, 4, 64, 128
H, W, Hl, Wl = 24, 24, 48, 48
HW = H * W            # 576
NLOW = T * Hl * Wl    # 9216
MTOT = Hl * Wl        # 2304
MBLK = MTOT // 4      # 576 query columns per core
NT = NLOW // 128      # 72 n-tiles
NPAIR = NT // 2       # 36 n-tile pairs for DoubleRow
NG = NT // 8          # 9 runt groups
HWP = 640             # 576 padded to 5*128
NC_CHUNKS = HWP // 128  # 5

SCALE_LOW = float(Ck) ** -0.5   # 0.125
BIAS_LOW = -2.5                 # constant logit shift; cancels in softmax
SCALE_HIGH = float(Cv) ** -0.5  # 0.0883883...

_PROGRAM = None
LAST_PERF = {}


def _u1d(n_in, n_out):
    """Half-pixel bilinear interpolation matrix (n_out, n_in), matches
    jax.image.resize(method='bilinear') for upsampling."""
    U = np.zeros((n_out, n_in), dtype=np.float64)
    scale = n_in / n_out
    for i in range(n_out):
        c = (i + 0.5) * scale - 0.5
        f = int(np.floor(c))
        frac = c - f
        lo = min(max(f, 0), n_in - 1)
        hi = min(max(f + 1, 0), n_in - 1)
        U[i, lo] += 1.0 - frac
        U[i, hi] += frac
    return U


def _build_upsample_full():
    """(H*W, Hl*Wl): column (ho*Wl+wo), row (h*W+w)."""
    Uh = _u1d(H, Hl)  # (48, 24)
    Uw = _u1d(W, Wl)  # (48, 24)
    Ufull = np.einsum("oh,pw->hwop", Uh, Uw).reshape(H * W, Hl * Wl)
    return Ufull.astype(np.float32)


def _build_program():
    nc = bacc.Bacc("TRN2", target_bir_lowering=False, debug=False)

    # qhead = qkl2 (576 cols) ++ mk tiles 0,1 (256 cols): one DMA gates QK(0)
    d_qhead = nc.dram_tensor("qhead", (128, MBLK + 256), BF16, kind="ExternalInput")
    # only the real 64 key rows come over DMA; the K-pad zeros (rows
    # 64-127) are memset once by the otherwise-idle Pool engine
    d_mk = nc.dram_tensor("mk", (64, NT, 128), BF16, kind="ExternalInput")
    d_mv8 = nc.dram_tensor("mv8", (128, NPAIR, 2, 128), F8, kind="ExternalInput")
    d_gkT = nc.dram_tensor("gkT", (128, T, NC_CHUNKS, Ck), BF16, kind="ExternalInput")
    d_gvT = nc.dram_tensor("gvT", (128, T, NC_CHUNKS, Cv), BF16, kind="ExternalInput")
    d_qvT = nc.dram_tensor("qvT", (128, NC_CHUNKS, Cv), BF16, kind="ExternalInput")
    d_uj = nc.dram_tensor("uj", (128, NC_CHUNKS, MBLK), BF16, kind="ExternalInput")
    d_out = nc.dram_tensor("out", (T * Ck + Cv, MBLK), F32, kind="ExternalOutput")

    EXP = mybir.ActivationFunctionType.Exp
    DR = mybir.MatmulPerfMode.DoubleRow

    with tile.TileContext(nc) as tc:
        from contextlib import ExitStack

        with ExitStack() as ctx:
            cp = ctx.enter_context(tc.tile_pool(name="const", bufs=1))
            wp = ctx.enter_context(tc.tile_pool(name="work", bufs=1))

            qhead_t = cp.tile([128, MBLK + 256], BF16)
            qkl2_t = qhead_t[:, 0:MBLK]
            mk_t = cp.tile([128, NT, 128], BF16)
            mv8_t = cp.tile([128, NPAIR, 2, 128], F8)
            gkT_t = cp.tile([128, T, NC_CHUNKS, Ck], BF16)
            gvT_t = cp.tile([128, T, NC_CHUNKS, Cv], BF16)
            qvT_t = cp.tile([128, NC_CHUNKS, Cv], BF16)
            uj_t = cp.tile([128, NC_CHUNKS, MBLK], BF16)

            # DMA order: gate the first QK as early as possible; every tile
            # must land before its loop trigger point (QK pace ~0.55us/tile,
            # DMA delivers ~10 tiles/us). qhead is split across two queues
            # so both halves transfer in parallel.
            # chunk A covers everything main(0) needs (qkl2[0:512] + tile 0)
            nc.sync.dma_start(qhead_t[:, 0:352], d_qhead.ap()[:, 0:352])
            nc.sync.dma_start(qhead_t[:, 352:704], d_qhead.ap()[:, 352:704])
            nc.sync.dma_start(qhead_t[:, 704:MBLK + 256],
                              d_qhead.ap()[:, 704:MBLK + 256])
            nc.sync.dma_start(mk_t[0:64, 2:12, :], d_mk.ap()[:, 2:12, :])
            nc.sync.dma_start(gkT_t[:], d_gkT.ap()[:, :, :, :])
            nc.sync.dma_start(mk_t[0:64, 12:20, :], d_mk.ap()[:, 12:20, :])
            nc.sync.dma_start(gvT_t[:], d_gvT.ap()[:, :, :, :])
            nc.sync.dma_start(qvT_t[:], d_qvT.ap()[:, :, :])
            nc.sync.dma_start(uj_t[:], d_uj.ap()[:, :, :])
            nc.sync.dma_start(mk_t[0:64, 20:32, :], d_mk.ap()[:, 20:32, :])
            nc.sync.dma_start(mv8_t[:, 0:18, :, :], d_mv8.ap()[:, 0:18, :, :])
            nc.sync.dma_start(mk_t[0:64, 32:72, :], d_mk.ap()[:, 32:72, :])
            nc.sync.dma_start(mv8_t[:, 18:36, :, :], d_mv8.ap()[:, 18:36, :, :])

            # small consts FIRST: bias_t gates the first exp and ones8_t
            # gates dn(0); the big K-pad memsets must queue behind them
            ones8_t = cp.tile([128, 2, 128], F8)
            nc.gpsimd.memset(ones8_t[:], 1.0)
            ones_bf = cp.tile([128, 128], BF16)
            nc.gpsimd.memset(ones_bf[:], 1.0)
            bias_t = cp.tile([128, 1], F32)
            nc.gpsimd.memset(bias_t[:], BIAS_LOW)

            # K-pad zeros, chunked so each lands before its tiles are read
            nc.gpsimd.memset(mk_t[64:128, 2:12, :], 0.0)
            nc.gpsimd.memset(mk_t[64:128, 12:32, :], 0.0)
            nc.gpsimd.memset(mk_t[64:128, 32:72, :], 0.0)

            # denominator runt accumulator (SBUF, fp32; DVE chain)
            racc = wp.tile([128, 8, 64], F32)

            # dummy exp: forces the ACT_TABLE_LOAD (~1.3us) to run now,
            # while the scalar engine idles on DMA, instead of right
            # before the first real exp
            warm_t = wp.tile([128, 1], F32)
            nc.scalar.activation(warm_t[:], bias_t[:], EXP)

            # ---------- high-branch emitters (interleaved into the loop) ----
            ea = []
            wts = []
            qvup_bf = wp.tile([128, MBLK], BF16)
            qops_cm = [None, None]  # (contextmanager, pool)

            def emit_ga():
                # all 4 per-frame attention tiles share one PSUM bank and
                # one exp activation (256 elems) instead of four
                with tc.tile_pool(name="hps", bufs=1, space="PSUM") as hps:
                    gat = hps.tile([128, T, Ck], F32, name="gat")
                    for t in range(T):
                        for c in range(NC_CHUNKS):
                            nc.tensor.matmul(
                                gat[:, t, :],
                                gvT_t[:, t, c, :],
                                gkT_t[:, t, c, :],
                                start=(c == 0),
                                stop=(c == NC_CHUNKS - 1),
                            )
                    ea_all = wp.tile([128, T, Ck], F32, name="ea_all")
                    nc.scalar.activation(ea_all[:], gat[:], EXP,
                                         scale=SCALE_HIGH)
                    ea.append(ea_all)

            def emit_high_softmax():
                ea_all = ea[0]
                s01 = wp.tile([128, Ck], F32)
                nc.vector.tensor_add(s01[:], ea_all[:, 0, :], ea_all[:, 1, :])
                s23 = wp.tile([128, Ck], F32)
                nc.vector.tensor_add(s23[:], ea_all[:, 2, :], ea_all[:, 3, :])
                ssum = wp.tile([128, Ck], F32)
                nc.vector.tensor_add(ssum[:], s01[:], s23[:])
                rs = wp.tile([128, Ck], F32)
                nc.vector.reciprocal(rs[:], ssum[:])
                for t in range(T):
                    wt = wp.tile([128, Ck], BF16, name=f"wt{t}", tag=f"wt{t}")
                    nc.vector.tensor_mul(wt[:], ea_all[:, t, :], rs[:])
                    wts.append(wt)

            def emit_qvup():
                with tc.tile_pool(name="qvups", bufs=1, space="PSUM") as qvups:
                    qvup = qvups.tile([128, MBLK], F32)
                    for c in range(NC_CHUNKS):
                        st, sp = (c == 0), (c == NC_CHUNKS - 1)
                        nc.tensor.matmul(
                            qvup[:, 0:512], qvT_t[:, c, :], uj_t[:, c, 0:512],
                            start=st, stop=sp,
                        )
                        nc.tensor.matmul(
                            qvup[:, 512:MBLK], qvT_t[:, c, :], uj_t[:, c, 512:MBLK],
                            start=st, stop=sp,
                        )
                    nc.vector.tensor_copy(qvup_bf[:], qvup[:])

            def emit_qo_t(t):
                # one qo per loop iteration; bufs=1 is stall-free because a
                # full iteration of qk work sits between consecutive qo's.
                if t == 0:
                    cm = tc.tile_pool(name="qops", bufs=1, space="PSUM")
                    qops_cm[0] = cm
                    qops_cm[1] = cm.__enter__()
                qops = qops_cm[1]
                qo = qops.tile([Ck, MBLK], F32, name=f"qo{t}", tag="qo")
                nc.tensor.matmul(
                    qo[:, 0:512], wts[t][:, :], qvup_bf[:, 0:512],
                    start=True, stop=True,
                )
                nc.tensor.matmul(
                    qo[:, 512:MBLK], wts[t][:, :], qvup_bf[:, 512:MBLK],
                    start=True, stop=True,
                )
                qo_sb = wp.tile([Ck, MBLK], F32, name=f"qosb{t}", tag="qosb")
                nc.vector.tensor_copy(qo_sb[:], qo[:])
                nc.sync.dma_start(
                    d_out.ap()[t * Ck:(t + 1) * Ck, :], qo_sb[:])
                if t == T - 1:
                    qops_cm[0].__exit__(None, None, None)

            # ================= low branch main loop =================
            # mains: pair-chunk QK psum (128, 2, 512) -> one 1024-elem exp.
            # runts: the 64-col leftovers of 8 consecutive tiles share one
            #        bank; bufs=2 so no trailing needed.
            # e_all: full fp8 attention matrix in SBUF; av consumes pairs
            #        via DoubleRow (K=256); Pool/DVE accumulate denominators.
            mains = ctx.enter_context(tc.tile_pool(name="mains", bufs=2, space="PSUM"))
            runts = ctx.enter_context(tc.tile_pool(name="runts", bufs=1, space="PSUM"))
            e_main = wp.tile([128, NT, 512], F8)
            e_runt = wp.tile([128, NT, 64], F8)

            mn = {}
            rn = [None]

            def lhsT_of(q):
                if q < 2:
                    return qhead_t[:, MBLK + q * 128:MBLK + (q + 1) * 128]
                return mk_t[:, q, :]

            def emit_qk_main(q):
                p = q // 2
                if q % 2 == 0:
                    mn[p] = mains.tile([128, 2, 512], F32, name=f"mn{p}", tag="mn")
                # high_priority: the scheduler's cost model underestimates
                # the DR consume stream and otherwise bunches it ahead of
                # the QK mains, starving the exp pipeline.
                with tc.high_priority():
                    nc.tensor.matmul(
                        mn[p][:, q % 2, :], lhsT_of(q),
                        qkl2_t[:, 0:512],
                        start=True, stop=True,
                    )

            def emit_qk_runt(q):
                g = q // 8
                if q % 8 == 0:
                    rn[0] = runts.tile([128, 8, 64], F32, name=f"rn{g}", tag="rn")
                nc.tensor.matmul(
                    rn[0][:, q % 8, :], lhsT_of(q),
                    qkl2_t[:, 512:MBLK],
                    start=True, stop=True,
                )

            def emit_exp_main(p):
                nc.scalar.activation(
                    e_main[:, 2 * p:2 * p + 2, :], mn[p][:, :, :], EXP,
                    scale=SCALE_LOW, bias=bias_t[:, 0:1])
                del mn[p]
                # dn rides right behind the exp: this spreads the ones-DR
                # ballast into the first half of the loop where the PE has
                # spare capacity (consumes only start at q=21)
                nc.tensor.matmul(dn[:, :], ones8_t[:, :, :],
                                 e_main[:, 2 * p:2 * p + 2, :],
                                 start=(p == 0), stop=(p == NPAIR - 1),
                                 perf_mode=DR)

            def emit_exp_runt(g):
                nc.scalar.activation(
                    e_runt[:, 8 * g:8 * g + 8, :], rn[0][:, :, :], EXP,
                    scale=SCALE_LOW, bias=bias_t[:, 0:1])
                # DVE accumulates the denominator runt part
                if g == 0:
                    nc.vector.tensor_copy(racc[:, :, :], e_runt[:, 0:8, :])
                else:
                    nc.vector.tensor_add(
                        racc[:, :, :], e_runt[:, 8 * g:8 * g + 8, :],
                        racc[:, :, :])

            av = None
            dn = None
            a64 = None

            nxt_av = [0]
            nxt_a64 = [0]

            def emit_av(p):
                nc.tensor.matmul(av[:, :], mv8_t[:, p, :, :],
                                 e_main[:, 2 * p:2 * p + 2, :],
                                 start=(p == 0), stop=(p == NPAIR - 1),
                                 perf_mode=DR)

            def emit_a64(p):
                nc.tensor.matmul(a64[:, :], mv8_t[:, p, :, :],
                                 e_runt[:, 2 * p:2 * p + 2, :],
                                 start=(p == 0), stop=(p == NPAIR - 1),
                                 perf_mode=DR)

            def pair_ready_iter(p):
                # the runt exp covering tile 2p+1 lands at iteration
                # 8g+11 (trail-4 runts); group 8 is pulled early to q=70.
                g = (2 * p + 1) // 8
                return 71 if g == 8 else 8 * g + 11 + 1

            def consume_ready(q, av_budget=2, a64_budget=1):
                # av tracks the exp stream 1:1 (it only needs e_main);
                # only the small a64 matmuls wait on runt-group exps.
                # Decoupling smooths the PE stream: readiness otherwise
                # arrives 4 pairs at a time and the bursts starve the
                # exp pipeline of QK mains.
                while (nxt_av[0] < NPAIR and av_budget > 0
                       and q >= 2 * nxt_av[0] + 3):
                    emit_av(nxt_av[0])
                    nxt_av[0] += 1
                    av_budget -= 1
                while (nxt_a64[0] < NPAIR and a64_budget > 0
                       and q >= pair_ready_iter(nxt_a64[0])):
                    emit_a64(nxt_a64[0])
                    nxt_a64[0] += 1
                    a64_budget -= 1

            rcp_sb = wp.tile([128, MBLK], F32)
            racc_bf = wp.tile([128, 64], BF16)

            dnps = ctx.enter_context(
                tc.tile_pool(name="dnps", bufs=1, space="PSUM"))
            dn = dnps.tile([128, 512], F32)

            for q in range(NT):
                emit_qk_main(q)
                if 4 <= q < 70:
                    emit_qk_runt(q - 4)
                if q % 2 == 1:
                    emit_exp_main(q // 2)
                if q >= 11 and (q - 11) % 8 == 0 and q < 70:
                    emit_exp_runt((q - 11) // 8)

                if q == 10:
                    emit_ga()
                elif q == 12:
                    emit_high_softmax()
                elif q == 14:
                    emit_qvup()
                elif 16 <= q <= 19:
                    emit_qo_t(q - 16)
                elif q == 20:
                    avps = ctx.enter_context(
                        tc.tile_pool(name="avps", bufs=1, space="PSUM"))
                    a64ps = ctx.enter_context(
                        tc.tile_pool(name="a64ps", bufs=1, space="PSUM"))
                    av = avps.tile([128, 512], F32)
                    a64 = a64ps.tile([128, 64], F32)
                elif q == 70:
                    # pull the whole runt-group-8 chain ahead of the last
                    # main exp so the DVE folds and the runt reciprocal
                    # complete while the PE drains dn/av
                    for r in range(66, NT):
                        emit_qk_runt(r)
                    emit_exp_runt(NG - 1)
                    nc.vector.tensor_add(
                        racc[:, 0:4, :], racc[:, 0:4, :], racc[:, 4:8, :])
                    nc.vector.tensor_add(
                        racc[:, 0:2, :], racc[:, 0:2, :], racc[:, 2:4, :])
                    nc.vector.tensor_add(
                        racc_bf[:, :], racc[:, 0, :], racc[:, 1, :])
                    # fold lands in the freed runts bank, NOT in a64 —
                    # sharing a64 would serialize the a64[0] AV drain
                    # behind this fold's reciprocal reader
                    rfold = runts.tile([128, 64], F32, name="rfold", tag="rn")
                    nc.tensor.matmul(rfold[:, :], ones_bf[:, :],
                                     racc_bf[:, :], start=True, stop=True)
                    nc.vector.reciprocal_approx_fast(
                        rcp_sb[:, 512:MBLK], rfold[:, :])
                if q >= 21 and q % 2 == 1:
                    consume_ready(q)

            # ---------------- drain ----------------
            # Denominators first: the 512-col reciprocal only needs the dn
            # matmuls, so it overlaps the AV drain instead of serializing.
            # The runt reciprocal already completed in-loop (q=70).
            mem_sb = wp.tile([128, 512], F32)
            mem_rt = wp.tile([128, 64], F32)
            r0 = T * Ck
            # dn chain completed in-loop (dn(p) rides behind each exp)
            nc.vector.reciprocal_approx_fast(rcp_sb[:, 0:512], dn[:, :])
            for p in range(nxt_av[0], NPAIR):
                emit_av(p)
            for p in range(nxt_a64[0], NPAIR):
                emit_a64(p)
            nc.vector.tensor_mul(
                mem_sb[:, 0:512], av[:, :], rcp_sb[:, 0:512])
            nc.sync.dma_start(
                d_out.ap()[r0:r0 + Cv, 0:512], mem_sb[:, 0:512])
            nc.vector.tensor_mul(
                mem_rt[:, :], a64[:, :], rcp_sb[:, 512:MBLK])
            nc.sync.dma_start(
                d_out.ap()[r0:r0 + Cv, 512:MBLK], mem_rt[:, :])

    nc.compile()
    return nc


def _get_program():
    global _PROGRAM
    if _PROGRAM is None:
        _PROGRAM = _build_program()
    return _PROGRAM


def _prep_core_inputs(memory_keys, memory_values, query_value,
                      memory_keys_low, memory_values_low, query_key_low,
                      Ufull, b, j):
    bf = ml_dtypes.bfloat16
    f8 = ml_dtypes.float8_e4m3

    # ---- low branch
    mk_cn = np.ascontiguousarray(
        memory_keys_low[b].transpose(1, 0, 2, 3).reshape(Ck, NLOW)
    )
    # only the 64 real key rows ship; the kernel memsets the K-pad zeros
    mk_send = np.ascontiguousarray(mk_cn.reshape(Ck, NT, 128)).astype(bf)

    mv_cn = memory_values_low[b].transpose(1, 0, 2, 3).reshape(Cv, NLOW)
    # (p, pair, slot, cv): slot s of pair p is n-tile 2p+s
    mv8 = np.ascontiguousarray(
        mv_cn.reshape(Cv, NPAIR, 2, 128).transpose(3, 1, 2, 0)
    ).astype(f8)

    qkl = query_key_low[b].reshape(Ck, MTOT)[:, j * MBLK:(j + 1) * MBLK]
    qkl2 = np.ascontiguousarray(np.concatenate([qkl, qkl], axis=0)).astype(bf)
    mk01_pad = np.zeros((128, 2, 128), np.float32).astype(bf)
    mk01_pad[:Ck] = mk_send[:, 0:2, :]
    qhead = np.ascontiguousarray(np.concatenate(
        [qkl2, mk01_pad.reshape(128, 256)], axis=1))

    # ---- high branch (zero-padded hw -> 640 = 5*128 chunks)
    gk = memory_keys[b].reshape(T, Ck, HW)
    gkp = np.zeros((T, Ck, HWP), np.float32)
    gkp[:, :, :HW] = gk
    gkT = np.ascontiguousarray(
        gkp.reshape(T, Ck, NC_CHUNKS, 128).transpose(3, 0, 2, 1)
    ).astype(bf)  # (p, t, c, k)

    gv = memory_values[b].reshape(T, Cv, HW)
    gvp = np.zeros((T, Cv, HWP), np.float32)
    gvp[:, :, :HW] = gv
    gvT = np.ascontiguousarray(
        gvp.reshape(T, Cv, NC_CHUNKS, 128).transpose(3, 0, 2, 1)
    ).astype(bf)  # (p, t, c, v)

    qv = query_value[b].reshape(Cv, HW)
    qvp = np.zeros((Cv, HWP), np.float32)
    qvp[:, :HW] = qv
    qvT = np.ascontiguousarray(
        qvp.reshape(Cv, NC_CHUNKS, 128).transpose(2, 1, 0)
    ).astype(bf)  # (p, c, v)

    ujf = np.zeros((HWP, MBLK), np.float32)
    ujf[:HW, :] = Ufull[:, j * MBLK:(j + 1) * MBLK]
    uj = np.ascontiguousarray(
        ujf.reshape(NC_CHUNKS, 128, MBLK).transpose(1, 0, 2)
    ).astype(bf)  # (p, c, o)

    return {
        "qhead": qhead, "mk": mk_send, "mv8": mv8,
        "gkT": gkT, "gvT": gvT, "qvT": qvT, "uj": uj,
    }


def kernel(memory_keys, memory_values, query_value,
           memory_keys_low, memory_values_low, query_key_low):
    memory_keys = np.asarray(memory_keys, dtype=np.float32)
    memory_values = np.asarray(memory_values, dtype=np.float32)
    query_value = np.asarray(query_value, dtype=np.float32)
    memory_keys_low = np.asarray(memory_keys_low, dtype=np.float32)
    memory_values_low = np.asarray(memory_values_low, dtype=np.float32)
    query_key_low = np.asarray(query_key_low, dtype=np.float32)

    Ufull = _build_upsample_full()
    nc = _get_program()

    in_maps = []
    for core in range(8):
        b, j = core // 4, core % 4
        in_maps.append(_prep_core_inputs(
            memory_keys, memory_values, query_value,
            memory_keys_low, memory_values_low, query_key_low, Ufull, b, j))

    trace = os.environ.get("KERNEL_TRACE", "0") == "1"
    kwargs = {}
    if trace and os.environ.get("KERNEL_TRACE_DIR"):
        os.makedirs(os.environ["KERNEL_TRACE_DIR"], exist_ok=True)
        kwargs["tmpdir"] = os.environ["KERNEL_TRACE_DIR"]
    res = bass_utils.run_bass_kernel_spmd(
        nc, in_maps, core_ids=list(range(8)), trace=trace, **kwargs
    )
    LAST_PERF.clear()
    LAST_PERF.update(
        exec_time_ns=res.exec_time_ns,
        mean_exec_time_ns=getattr(res, "mean_exec_time_ns", None),
        max_exec_time_core_id=getattr(res, "max_exec_time_core_id", None),
        per_core_scope_times=getattr(res, "per_core_scope_times", None),
        trace=getattr(res, "instructions_and_trace", None),
    )

    out = np.empty((B, T * Ck + Cv, Hl, Wl), np.float32)
    for core in range(8):
        b, j = core // 4, core % 4
        blk = res.results[core]["out"]  # (384, 576)
        out[b, :, 12 * j:12 * (j + 1), :] = blk.reshape(T * Ck + Cv, 12, Wl)
    return out


# revision 52
# speedup vs baseline: 1.0188x; 1.0188x over previous
"""Trainium2 Bass kernel for nn_MemoryModule (sparse_attention).

Reference computation (per batch b):
  Low branch:
    mkl (9216, 64) = memory_keys_low[b] as (T*Hl*Wl, Ck)
    qkl (64, 2304) = query_key_low[b]
    A = softmax_over_n(mkl @ qkl * Ck^-0.5)          # (9216, 2304)
    memory = mvl @ A                                  # (128, 2304), mvl = (Cv, T*Hl*Wl)
  High branch:
    g_attn[t] = softmax_over_t(gk[t] @ gv[t].T * Cv^-0.5)   # (Ck, Cv) per t
    qout[t] = g_attn[t] @ qv                          # (64, 576) -> (256, 24, 24)
    qout = bilinear_upsample_2x(qout)                 # (256, 48, 48)
  out = concat([qout, memory.reshape(128, 48, 48)])   # (384, 48, 48)

Sharding: 8 cores = (b in 0..1) x (j in 0..3), where j picks 576 of the 2304
low-branch query columns (= 12 of the 48 output rows). Softmax is over the
key axis, so column blocks are independent -> no collectives.

Design notes (evolved from the v1 pipeline via trace analysis):
 - QK lhsT is zero-padded from K=64 to K=128 so every matmul in the kernel
   runs in the same (128,128) PE mode: no row-group tiling mode switches,
   and the compiler's Fast Weight Load stays enabled for the QK stream.
 - The softmax denominator main part (512 cols) runs on the PE as
   DoubleRow ones-matmuls, each emitted directly behind its exp: this
   spreads the ballast into the first half of the loop (where the PE has
   spare capacity) and keeps the HAM clock gate warm from ~16us on
   (cold PE = 1.2 GHz, warm = 2.4 GHz). Only the runt part (64 cols)
   accumulates on the DVE (9 contiguous adds); elementwise engines
   measured 1.2-3 ns/elem - far too slow for the 37k-element main sum.
 - The attention matrix is stored as TWO tensors, e_main (128,72,512)
   and e_runt (128,72,64), so every exp write, every AV DoubleRow rhs
   and the DVE accumulation slices are fully contiguous.
 - exp() writes fp8-e4m3 (constant logit shift -2.5 cancels in softmax).
 - QK results land in PSUM as pair chunks (128,2,512) -> one activation
   covers 1024 elements (ACT costs (N+352)/1.2 ns, so batching matters);
   64-col leftovers of 8 tiles share one bank (runt matmuls trail the
   mains by 4 tiles so bufs=1 never stalls).
 - QK mains are emitted under tc.high_priority(): the scheduler's cost
   model underestimates the DR consume stream and otherwise bunches it
   ahead of the mains, starving the exp pipeline for ~6us.
 - The runt-group-8 chain (QK runts, exp, DVE folds, partition-fold
   matmul, runt reciprocal) is pulled into the loop at q=70, ahead of
   the last main exp, so it overlaps the dn/av drains. The fold lands
   in the freed runts bank - sharing a PSUM tile with the a64 AV chain
   would serialize the drain behind the reciprocal reader (~1.5us).
 - AV consumes fp8 n-tile PAIRS via MatmulPerfMode.DoubleRow (K=256).
 - PSUM budget stays exactly 8 banks in every phase:
   mains 2x2 + runts 1 + dn 1 + {hps 1 | qvups 2 | qops 2 | av 1 + a64 1}.
"""

import os
import sys

for _p in ("/opt/trn_rl_repo",):
    if _p not in sys.path and os.path.isdir(_p):
        sys.path.insert(0, _p)

import numpy as np
import ml_dtypes

import concourse.bass as bass
import concourse.tile as tile
from concourse import bacc, mybir
from concourse import bass_utils

BF16 = mybir.dt.bfloat16
F32 = mybir.dt.float32
F8 = mybir.dt.float8e4

B, T, Ck, Cv = 2, 4, 64, 128
H, W, Hl, Wl = 24, 24, 48, 48
HW = H * W            # 576
NLOW = T * Hl * Wl    # 9216
MTOT = Hl * Wl        # 2304
MBLK = MTOT // 4      # 576 query columns per core
NT = NLOW // 128      # 72 n-tiles
NPAIR = NT // 2       # 36 n-tile pairs for DoubleRow
NG = NT // 8          # 9 runt groups
HWP = 640             # 576 padded to 5*128
NC_CHUNKS = HWP // 128  # 5

SCALE_LOW = float(Ck) ** -0.5   # 0.125
BIAS_LOW = -2.5                 # constant logit shift; cancels in softmax
SCALE_HIGH = float(Cv) ** -0.5  # 0.0883883...

_PROGRAM = None
LAST_PERF = {}


def _u1d(n_in, n_out):
    """Half-pixel bilinear interpolation matrix (n_out, n_in), matches
    jax.image.resize(method='bilinear') for upsampling."""
    U = np.zeros((n_out, n_in), dtype=np.float64)
    scale = n_in / n_out
    for i in range(n_out):
        c = (i + 0.5) * scale - 0.5
        f = int(np.floor(c))
        frac = c - f
        lo = min(max(f, 0), n_in - 1)
        hi = min(max(f + 1, 0), n_in - 1)
        U[i, lo] += 1.0 - frac
        U[i, hi] += frac
    return U


def _build_upsample_full():
    """(H*W, Hl*Wl): column (ho*Wl+wo), row (h*W+w)."""
    Uh = _u1d(H, Hl)  # (48, 24)
    Uw = _u1d(W, Wl)  # (48, 24)
    Ufull = np.einsum("oh,pw->hwop", Uh, Uw).reshape(H * W, Hl * Wl)
    return Ufull.astype(np.float32)


def _build_program():
    nc = bacc.Bacc("TRN2", target_bir_lowering=False, debug=False)

    # qhead = qkl2 (576 cols) ++ mk tiles 0,1 (256 cols): one DMA gates QK(0)
    d_qhead = nc.dram_tensor("qhead", (128, MBLK + 256), BF16, kind="ExternalInput")
    # only the real 64 key rows come over DMA; the K-pad zeros (rows
    # 64-127) are memset once by the otherwise-idle Pool engine
    d_mk = nc.dram_tensor("mk", (64, NT, 128), BF16, kind="ExternalInput")
    d_mv8 = nc.dram_tensor("mv8", (128, NPAIR, 2, 128), F8, kind="ExternalInput")
    d_gkT = nc.dram_tensor("gkT", (128, T, NC_CHUNKS, Ck), BF16, kind="ExternalInput")
    d_gvT = nc.dram_tensor("gvT", (128, T, NC_CHUNKS, Cv), BF16, kind="ExternalInput")
    d_qvT = nc.dram_tensor("qvT", (128, NC_CHUNKS, Cv), BF16, kind="ExternalInput")
    d_uj = nc.dram_tensor("uj", (128, NC_CHUNKS, MBLK), BF16, kind="ExternalInput")
    d_out = nc.dram_tensor("out", (T * Ck + Cv, MBLK), F32, kind="ExternalOutput")

    EXP = mybir.ActivationFunctionType.Exp
    DR = mybir.MatmulPerfMode.DoubleRow

    with tile.TileContext(nc) as tc:
        from contextlib import ExitStack

        with ExitStack() as ctx:
            cp = ctx.enter_context(tc.tile_pool(name="const", bufs=1))
            wp = ctx.enter_context(tc.tile_pool(name="work", bufs=1))

            qhead_t = cp.tile([128, MBLK + 256], BF16)
            qkl2_t = qhead_t[:, 0:MBLK]
            mk_t = cp.tile([128, NT, 128], BF16)
            mv8_t = cp.tile([128, NPAIR, 2, 128], F8)
            gkT_t = cp.tile([128, T, NC_CHUNKS, Ck], BF16)
            gvT_t = cp.tile([128, T, NC_CHUNKS, Cv], BF16)
            qvT_t = cp.tile([128, NC_CHUNKS, Cv], BF16)
            uj_t = cp.tile([128, NC_CHUNKS, MBLK], BF16)

            # DMA order: gate the first QK as early as possible; every tile
            # must land before its loop trigger point (QK pace ~0.55us/tile,
            # DMA delivers ~10 tiles/us). qhead is split across two queues
            # so both halves transfer in parallel.
            # chunk A covers everything main(0) needs (qkl2[0:512] + tile 0)
            nc.sync.dma_start(qhead_t[:, 0:704], d_qhead.ap()[:, 0:704])
            nc.sync.dma_start(qhead_t[:, 704:MBLK + 256],
                              d_qhead.ap()[:, 704:MBLK + 256])
            nc.sync.dma_start(mk_t[0:64, 2:12, :], d_mk.ap()[:, 2:12, :])
            nc.sync.dma_start(gkT_t[:], d_gkT.ap()[:, :, :, :])
            nc.sync.dma_start(mk_t[0:64, 12:20, :], d_mk.ap()[:, 12:20, :])
            nc.sync.dma_start(gvT_t[:], d_gvT.ap()[:, :, :, :])
            nc.sync.dma_start(qvT_t[:], d_qvT.ap()[:, :, :])
            nc.sync.dma_start(uj_t[:], d_uj.ap()[:, :, :])
            nc.sync.dma_start(mk_t[0:64, 20:32, :], d_mk.ap()[:, 20:32, :])
            nc.sync.dma_start(mv8_t[:, 0:18, :, :], d_mv8.ap()[:, 0:18, :, :])
            nc.sync.dma_start(mk_t[0:64, 32:72, :], d_mk.ap()[:, 32:72, :])
            nc.sync.dma_start(mv8_t[:, 18:36, :, :], d_mv8.ap()[:, 18:36, :, :])

            # small consts FIRST: bias_t gates the first exp and ones8_t
            # gates dn(0); the big K-pad memsets must queue behind them
            ones8_t = cp.tile([128, 2, 128], F8)
            nc.gpsimd.memset(ones8_t[:], 1.0)
            ones_bf = cp.tile([128, 128], BF16)
            nc.gpsimd.memset(ones_bf[:], 1.0)
            bias_t = cp.tile([128, 1], F32)
            nc.gpsimd.memset(bias_t[:], BIAS_LOW)

            # K-pad zeros, chunked so each lands before its tiles are read
            nc.gpsimd.memset(mk_t[64:128, 2:12, :], 0.0)
            nc.gpsimd.memset(mk_t[64:128, 12:32, :], 0.0)
            nc.gpsimd.memset(mk_t[64:128, 32:72, :], 0.0)

            # denominator runt accumulator (SBUF, fp32; DVE chain)
            racc = wp.tile([128, 8, 64], F32)

            # dummy exp: forces the ACT_TABLE_LOAD (~1.3us) to run now,
            # while the scalar engine idles on DMA, instead of right
            # before the first real exp
            warm_t = wp.tile([128, 1], F32)
            nc.scalar.activation(warm_t[:], bias_t[:], EXP)

            # ---------- high-branch emitters (interleaved into the loop) ----
            ea = []
            wts = []
            qvup_bf = wp.tile([128, MBLK], BF16)
            qops_cm = [None, None]  # (contextmanager, pool)

            def emit_ga():
                # all 4 per-frame attention tiles share one PSUM bank and
                # one exp activation (256 elems) instead of four
                with tc.tile_pool(name="hps", bufs=1, space="PSUM") as hps:
                    gat = hps.tile([128, T, Ck], F32, name="gat")
                    for t in range(T):
                        for c in range(NC_CHUNKS):
                            nc.tensor.matmul(
                                gat[:, t, :],
                                gvT_t[:, t, c, :],
                                gkT_t[:, t, c, :],
                                start=(c == 0),
                                stop=(c == NC_CHUNKS - 1),
                            )
                    ea_all = wp.tile([128, T, Ck], F32, name="ea_all")
                    nc.scalar.activation(ea_all[:], gat[:], EXP,
                                         scale=SCALE_HIGH)
                    ea.append(ea_all)

            def emit_high_softmax():
                ea_all = ea[0]
                s01 = wp.tile([128, Ck], F32)
                nc.vector.tensor_add(s01[:], ea_all[:, 0, :], ea_all[:, 1, :])
                s23 = wp.tile([128, Ck], F32)
                nc.vector.tensor_add(s23[:], ea_all[:, 2, :], ea_all[:, 3, :])
                ssum = wp.tile([128, Ck], F32)
                nc.vector.tensor_add(ssum[:], s01[:], s23[:])
                rs = wp.tile([128, Ck], F32)
                nc.vector.reciprocal(rs[:], ssum[:])
                for t in range(T):
                    wt = wp.tile([128, Ck], BF16, name=f"wt{t}", tag=f"wt{t}")
                    nc.vector.tensor_mul(wt[:], ea_all[:, t, :], rs[:])
                    wts.append(wt)

            def emit_qvup():
                with tc.tile_pool(name="qvups", bufs=1, space="PSUM") as qvups:
                    qvup = qvups.tile([128, MBLK], F32)
                    for c in range(NC_CHUNKS):
                        st, sp = (c == 0), (c == NC_CHUNKS - 1)
                        nc.tensor.matmul(
                            qvup[:, 0:512], qvT_t[:, c, :], uj_t[:, c, 0:512],
                            start=st, stop=sp,
                        )
                        nc.tensor.matmul(
                            qvup[:, 512:MBLK], qvT_t[:, c, :], uj_t[:, c, 512:MBLK],
                            start=st, stop=sp,
                        )
                    nc.vector.tensor_copy(qvup_bf[:], qvup[:])

            def emit_qo_t(t):
                # one qo per loop iteration; bufs=1 is stall-free because a
                # full iteration of qk work sits between consecutive qo's.
                if t == 0:
                    cm = tc.tile_pool(name="qops", bufs=1, space="PSUM")
                    qops_cm[0] = cm
                    qops_cm[1] = cm.__enter__()
                qops = qops_cm[1]
                qo = qops.tile([Ck, MBLK], F32, name=f"qo{t}", tag="qo")
                nc.tensor.matmul(
                    qo[:, 0:512], wts[t][:, :], qvup_bf[:, 0:512],
                    start=True, stop=True,
                )
                nc.tensor.matmul(
                    qo[:, 512:MBLK], wts[t][:, :], qvup_bf[:, 512:MBLK],
                    start=True, stop=True,
                )
                qo_sb = wp.tile([Ck, MBLK], F32, name=f"qosb{t}", tag="qosb")
                nc.vector.tensor_copy(qo_sb[:], qo[:])
                nc.sync.dma_start(
                    d_out.ap()[t * Ck:(t + 1) * Ck, :], qo_sb[:])
                if t == T - 1:
                    qops_cm[0].__exit__(None, None, None)

            # ================= low branch main loop =================
            # mains: pair-chunk QK psum (128, 2, 512) -> one 1024-elem exp.
            # runts: the 64-col leftovers of 8 consecutive tiles share one
            #        bank; bufs=2 so no trailing needed.
            # e_all: full fp8 attention matrix in SBUF; av consumes pairs
            #        via DoubleRow (K=256); Pool/DVE accumulate denominators.
            mains = ctx.enter_context(tc.tile_pool(name="mains", bufs=2, space="PSUM"))
            runts = ctx.enter_context(tc.tile_pool(name="runts", bufs=1, space="PSUM"))
            e_main = wp.tile([128, NT, 512], F8)
            e_runt = wp.tile([128, NT, 64], F8)

            mn = {}
            rn = [None]

            def lhsT_of(q):
                if q < 2:
                    return qhead_t[:, MBLK + q * 128:MBLK + (q + 1) * 128]
                return mk_t[:, q, :]

            def emit_qk_main(q):
                p = q // 2
                if q % 2 == 0:
                    mn[p] = mains.tile([128, 2, 512], F32, name=f"mn{p}", tag="mn")
                # high_priority: the scheduler's cost model underestimates
                # the DR consume stream and otherwise bunches it ahead of
                # the QK mains, starving the exp pipeline.
                with tc.high_priority():
                    nc.tensor.matmul(
                        mn[p][:, q % 2, :], lhsT_of(q),
                        qkl2_t[:, 0:512],
                        start=True, stop=True,
                    )

            def emit_qk_runt(q):
                g = q // 8
                if q % 8 == 0:
                    rn[0] = runts.tile([128, 8, 64], F32, name=f"rn{g}", tag="rn")
                nc.tensor.matmul(
                    rn[0][:, q % 8, :], lhsT_of(q),
                    qkl2_t[:, 512:MBLK],
                    start=True, stop=True,
                )

            def emit_exp_main(p):
                nc.scalar.activation(
                    e_main[:, 2 * p:2 * p + 2, :], mn[p][:, :, :], EXP,
                    scale=SCALE_LOW, bias=bias_t[:, 0:1])
                del mn[p]
                # dn rides right behind the exp: this spreads the ones-DR
                # ballast into the first half of the loop where the PE has
                # spare capacity (consumes only start at q=21)
                nc.tensor.matmul(dn[:, :], ones8_t[:, :, :],
                                 e_main[:, 2 * p:2 * p + 2, :],
                                 start=(p == 0), stop=(p == NPAIR - 1),
                                 perf_mode=DR)

            def emit_exp_runt(g):
                nc.scalar.activation(
                    e_runt[:, 8 * g:8 * g + 8, :], rn[0][:, :, :], EXP,
                    scale=SCALE_LOW, bias=bias_t[:, 0:1])
                # DVE accumulates the denominator runt part
                if g == 0:
                    nc.vector.tensor_copy(racc[:, :, :], e_runt[:, 0:8, :])
                else:
                    nc.vector.tensor_add(
                        racc[:, :, :], e_runt[:, 8 * g:8 * g + 8, :],
                        racc[:, :, :])

            av = None
            dn = None
            a64 = None

            nxt_av = [0]
            nxt_a64 = [0]

            def emit_av(p):
                nc.tensor.matmul(av[:, :], mv8_t[:, p, :, :],
                                 e_main[:, 2 * p:2 * p + 2, :],
                                 start=(p == 0), stop=(p == NPAIR - 1),
                                 perf_mode=DR)

            def emit_a64(p):
                nc.tensor.matmul(a64[:, :], mv8_t[:, p, :, :],
                                 e_runt[:, 2 * p:2 * p + 2, :],
                                 start=(p == 0), stop=(p == NPAIR - 1),
                                 perf_mode=DR)

            def pair_ready_iter(p):
                # the runt exp covering tile 2p+1 lands at iteration
                # 8g+11 (trail-4 runts); group 8 is pulled early to q=70.
                g = (2 * p + 1) // 8
                return 71 if g == 8 else 8 * g + 11 + 1

            def consume_ready(q, av_budget=2, a64_budget=1):
                # av tracks the exp stream 1:1 (it only needs e_main);
                # only the small a64 matmuls wait on runt-group exps.
                # Decoupling smooths the PE stream: readiness otherwise
                # arrives 4 pairs at a time and the bursts starve the
                # exp pipeline of QK mains.
                while (nxt_av[0] < NPAIR and av_budget > 0
                       and q >= 2 * nxt_av[0] + 3):
                    emit_av(nxt_av[0])
                    nxt_av[0] += 1
                    av_budget -= 1
                while (nxt_a64[0] < NPAIR and a64_budget > 0
                       and q >= pair_ready_iter(nxt_a64[0])):
                    emit_a64(nxt_a64[0])
                    nxt_a64[0] += 1
                    a64_budget -= 1

            rcp_sb = wp.tile([128, MBLK], F32)
            racc_bf = wp.tile([128, 64], BF16)

            dnps = ctx.enter_context(
                tc.tile_pool(name="dnps", bufs=1, space="PSUM"))
            dn = dnps.tile([128, 512], F32)

            for q in range(NT):
                emit_qk_main(q)
                if 4 <= q < 70:
                    emit_qk_runt(q - 4)
                if q % 2 == 1:
                    emit_exp_main(q // 2)
                if q >= 11 and (q - 11) % 8 == 0 and q < 70:
                    emit_exp_runt((q - 11) // 8)

                if q == 10:
                    emit_ga()
                elif q == 12:
                    emit_high_softmax()
                elif q == 14:
                    emit_qvup()
                elif 16 <= q <= 19:
                    emit_qo_t(q - 16)
                elif q == 20:
                    avps = ctx.enter_context(
                        tc.tile_pool(name="avps", bufs=1, space="PSUM"))
                    a64ps = ctx.enter_context(
                        tc.tile_pool(name="a64ps", bufs=1, space="PSUM"))
                    av = avps.tile([128, 512], F32)
                    a64 = a64ps.tile([128, 64], F32)
                elif q == 70:
                    # pull the whole runt-group-8 chain ahead of the last
                    # main exp so the DVE folds and the runt reciprocal
                    # complete while the PE drains dn/av
                    for r in range(66, NT):
                        emit_qk_runt(r)
                    emit_exp_runt(NG - 1)
                    nc.vector.tensor_add(
                        racc[:, 0:4, :], racc[:, 0:4, :], racc[:, 4:8, :])
                    nc.vector.tensor_add(
                        racc[:, 0:2, :], racc[:, 0:2, :], racc[:, 2:4, :])
                    nc.vector.tensor_add(
                        racc_bf[:, :], racc[:, 0, :], racc[:, 1, :])
                    # fold lands in the freed runts bank, NOT in a64 —
                    # sharing a64 would serialize the a64[0] AV drain
                    # behind this fold's reciprocal reader
                    rfold = runts.tile([128, 64], F32, name="rfold", tag="rn")
                    nc.tensor.matmul(rfold[:, :], ones_bf[:, :],
                                     racc_bf[:, :], start=True, stop=True)
                    nc.vector.reciprocal_approx_fast(
                        rcp_sb[:, 512:MBLK], rfold[:, :])
                if q >= 21 and q % 2 == 1:
                    consume_ready(q)

            # ---------------- drain ----------------
            # Denominators first: the 512-col reciprocal only needs the dn
            # matmuls, so it overlaps the AV drain instead of serializing.
            # The runt reciprocal already completed in-loop (q=70).
            mem_sb = wp.tile([128, 512], F32)
            mem_rt = wp.tile([128, 64], F32)
            r0 = T * Ck
            # dn chain completed in-loop (dn(p) rides behind each exp)
            nc.vector.reciprocal_approx_fast(rcp_sb[:, 0:512], dn[:, :])
            for p in range(nxt_av[0], NPAIR):
                emit_av(p)
            for p in range(nxt_a64[0], NPAIR):
                emit_a64(p)
            nc.vector.tensor_mul(
                mem_sb[:, 0:512], av[:, :], rcp_sb[:, 0:512])
            nc.sync.dma_start(
                d_out.ap()[r0:r0 + Cv, 0:512], mem_sb[:, 0:512])
            nc.vector.tensor_mul(
                mem_rt[:, :], a64[:, :], rcp_sb[:, 512:MBLK])
            nc.sync.dma_start(
                d_out.ap()[r0:r0 + Cv, 512:MBLK], mem_rt[:, :])

    nc.compile()
    return nc


def _get_program():
    global _PROGRAM
    if _PROGRAM is None:
        _PROGRAM = _build_program()
    return _PROGRAM


def _prep_core_inputs(memory_keys, memory_values, query_value,
                      memory_keys_low, memory_values_low, query_key_low,
                      Ufull, b, j):
    bf = ml_dtypes.bfloat16
    f8 = ml_dtypes.float8_e4m3

    # ---- low branch
    mk_cn = np.ascontiguousarray(
        memory_keys_low[b].transpose(1, 0, 2, 3).reshape(Ck, NLOW)
    )
    # only the 64 real key rows ship; the kernel memsets the K-pad zeros
    mk_send = np.ascontiguousarray(mk_cn.reshape(Ck, NT, 128)).astype(bf)

    mv_cn = memory_values_low[b].transpose(1, 0, 2, 3).reshape(Cv, NLOW)
    # (p, pair, slot, cv): slot s of pair p is n-tile 2p+s
    mv8 = np.ascontiguousarray(
        mv_cn.reshape(Cv, NPAIR, 2, 128).transpose(3, 1, 2, 0)
    ).astype(f8)

    qkl = query_key_low[b].reshape(Ck, MTOT)[:, j * MBLK:(j + 1) * MBLK]
    qkl2 = np.ascontiguousarray(np.concatenate([qkl, qkl], axis=0)).astype(bf)
    mk01_pad = np.zeros((128, 2, 128), np.float32).astype(bf)
    mk01_pad[:Ck] = mk_send[:, 0:2, :]
    qhead = np.ascontiguousarray(np.concatenate(
        [qkl2, mk01_pad.reshape(128, 256)], axis=1))

    # ---- high branch (zero-padded hw -> 640 = 5*128 chunks)
    gk = memory_keys[b].reshape(T, Ck, HW)
    gkp = np.zeros((T, Ck, HWP), np.float32)
    gkp[:, :, :HW] = gk
    gkT = np.ascontiguousarray(
        gkp.reshape(T, Ck, NC_CHUNKS, 128).transpose(3, 0, 2, 1)
    ).astype(bf)  # (p, t, c, k)

    gv = memory_values[b].reshape(T, Cv, HW)
    gvp = np.zeros((T, Cv, HWP), np.float32)
    gvp[:, :, :HW] = gv
    gvT = np.ascontiguousarray(
        gvp.reshape(T, Cv, NC_CHUNKS, 128).transpose(3, 0, 2, 1)
    ).astype(bf)  # (p, t, c, v)

    qv = query_value[b].reshape(Cv, HW)
    qvp = np.zeros((Cv, HWP), np.float32)
    qvp[:, :HW] = qv
    qvT = np.ascontiguousarray(
        qvp.reshape(Cv, NC_CHUNKS, 128).transpose(2, 1, 0)
    ).astype(bf)  # (p, c, v)

    ujf = np.zeros((HWP, MBLK), np.float32)
    ujf[:HW, :] = Ufull[:, j * MBLK:(j + 1) * MBLK]
    uj = np.ascontiguousarray(
        ujf.reshape(NC_CHUNKS, 128, MBLK).transpose(1, 0, 2)
    ).astype(bf)  # (p, c, o)

    return {
        "qhead": qhead, "mk": mk_send, "mv8": mv8,
        "gkT": gkT, "gvT": gvT, "qvT": qvT, "uj": uj,
    }


def kernel(memory_keys, memory_values, query_value,
           memory_keys_low, memory_values_low, query_key_low):
    memory_keys = np.asarray(memory_keys, dtype=np.float32)
    memory_values = np.asarray(memory_values, dtype=np.float32)
    query_value = np.asarray(query_value, dtype=np.float32)
    memory_keys_low = np.asarray(memory_keys_low, dtype=np.float32)
    memory_values_low = np.asarray(memory_values_low, dtype=np.float32)
    query_key_low = np.asarray(query_key_low, dtype=np.float32)

    Ufull = _build_upsample_full()
    nc = _get_program()

    in_maps = []
    for core in range(8):
        b, j = core // 4, core % 4
        in_maps.append(_prep_core_inputs(
            memory_keys, memory_values, query_value,
            memory_keys_low, memory_values_low, query_key_low, Ufull, b, j))

    trace = os.environ.get("KERNEL_TRACE", "0") == "1"
    kwargs = {}
    if trace and os.environ.get("KERNEL_TRACE_DIR"):
        os.makedirs(os.environ["KERNEL_TRACE_DIR"], exist_ok=True)
        kwargs["tmpdir"] = os.environ["KERNEL_TRACE_DIR"]
    res = bass_utils.run_bass_kernel_spmd(
        nc, in_maps, core_ids=list(range(8)), trace=trace, **kwargs
    )
    LAST_PERF.clear()
    LAST_PERF.update(
        exec_time_ns=res.exec_time_ns,
        mean_exec_time_ns=getattr(res, "mean_exec_time_ns", None),
        max_exec_time_core_id=getattr(res, "max_exec_time_core_id", None),
        per_core_scope_times=getattr(res, "per_core_scope_times", None),
        trace=getattr(res, "instructions_and_trace", None),
    )

    out = np.empty((B, T * Ck + Cv, Hl, Wl), np.float32)
    for core in range(8):
        b, j = core // 4, core % 4
        blk = res.results[core]["out"]  # (384, 576)
        out[b, :, 12 * j:12 * (j + 1), :] = blk.reshape(T * Ck + Cv, 12, Wl)
    return out
